# revision 1
# baseline (speedup 1.0000x reference)
"""Causal self-attention (B=4, T=2048, C=1024, H=16, D=64, RoPE) on 8 trn2 cores.

Sharding: data-parallel over batch (4) x tensor-parallel over head-halves (2).
core = 2*b + hh handles batch b, heads [hh*8, hh*8+8).

Per-core kernel (all matmuls bf16 with fp32 PSUM accumulation; every matmul
operand at partition base 0 — mixed PE tile positions fault on this setup):
  - QT/KT projection in transposed layout [c_out, t] (lhsT = W column block,
    rhs = x^T), RoPE via PE rotate-permutation matmul + DVE mul/add.
    Tiles hold head pairs: rows 0-63 head 2i, rows 64-127 head 2i+1.
  - V projection in natural layout [t, c_out], stored interleaved with a ones
    column per head (65 cols/head) for free softmax row-sums.
  - scores^T per head pair via ONE K=128 matmul: lhsT = K^T pair chunk
    [128d, 128k], rhs = block-diagonal assembled Q chunk [128, 512]
    (head A rows 0-63 cols 0-255, head B rows 64-127 cols 256-511, zeros
    elsewhere) -> scores^T [128 k, 256 qA | 256 qB].
  - exp on ACT without max subtraction (scores are O(10)); causal masking via
    multiplicative bf16 masks on the two diagonal key chunks.
  - PV: out_h^T accumulated over key chunks with lhsT = V'_h (ones column ->
    row 64 = softmax denominators); per-head psum bank so accumulation
    groups never share a zero region.
  - normalize Y^T by broadcast(1/sums) (PE outer-product), then row-parallel
    output projection -> partial [T, C] fp32 output.
Host sums the two partial outputs of each batch pair.
"""

import os

import numpy as np
import ml_dtypes

import concourse.bass as bass
import concourse.mybir as mybir
import concourse.tile as tile
from concourse.bass_utils import run_bass_kernel_spmd

BF16 = mybir.dt.bfloat16
F32 = mybir.dt.float32
NP_BF16 = ml_dtypes.bfloat16

B, T, C = 4, 2048, 1024
H, D = 16, 64
HPC = 8          # heads per core
CPC = HPC * D    # 512 features per core
N_CORES = 8
QC = 256         # query chunk (scores^T free dim per head)
KC = 128         # key chunk (scores^T partition dim)
NQC = T // QC    # 8 query chunks
ROPE_BASE = 10000.0

LAST_EXEC_NS = None
LAST_RESULTS = None


def _split_sync_waits(nc):
    """This walrus build accepts at most one sync wait per instruction; hoist
    extra waits onto same-engine NOPs inserted immediately before."""
    ctr = 0
    for bb in nc.main_func.blocks:
        insts = bb.instructions
        new = []
        changed = False
        for inst in insts:
            si = inst.sync_info
            waits = list(si.on_wait or []) if si is not None else []
            if len(waits) > 1:
                changed = True
                for w in waits[:-1]:
                    ctr += 1
                    nop = mybir.InstNoOp(
                        name=f"waitsplit_nop_{ctr}", ins=[], outs=[],
                        engine=inst.engine,
                    )
                    nop.sync_info = mybir.SyncInfo(on_wait=[w], on_update=[])
                    new.append(nop)
                inst.sync_info = mybir.SyncInfo(
                    on_wait=[waits[-1]], on_update=list(si.on_update or [])
                )
            new.append(inst)
        if changed:
            insts[:] = new


def _build_nc(split_waits=True, phases=3, attn_g=2, attn_qc=None):
    if attn_qc is None:
        attn_qc = NQC
    nc = bass.Bass()

    xT = nc.dram_tensor("xT", [C, T], BF16, kind="ExternalInput")
    wq = nc.dram_tensor("wq", [C, CPC], BF16, kind="ExternalInput")
    wk = nc.dram_tensor("wk", [C, CPC], BF16, kind="ExternalInput")
    wv = nc.dram_tensor("wv", [C, CPC], BF16, kind="ExternalInput")
    wc = nc.dram_tensor("wc", [CPC, C], BF16, kind="ExternalInput")
    cos2 = nc.dram_tensor("cos2", [128, T], BF16, kind="ExternalInput")
    ssin = nc.dram_tensor("ssin", [128, T], BF16, kind="ExternalInput")
    p128 = nc.dram_tensor("p128", [128, 128], BF16, kind="ExternalInput")
    # multiplicative causal masks for the 4-head-wide probs tile
    mska = nc.dram_tensor("mska", [128, 4 * QC], BF16, kind="ExternalInput")
    mskb = nc.dram_tensor("mskb", [128, 4 * QC], BF16, kind="ExternalInput")
    e2 = nc.dram_tensor("e2", [2, 128], BF16, kind="ExternalInput")
    out = nc.dram_tensor("out", [T, C], F32, kind="ExternalOutput")

    KB = C // 128          # 8 k-blocks over c_in
    NT = CPC // 128        # 4 head-pair tiles for QT/KT/YT
    TT16 = T // 128        # 16 t tiles for V

    with tile.TileContext(nc) as tc:
        with (
            tc.tile_pool(name="singles", bufs=1) as singles,
            tc.tile_pool(name="xw", bufs=1) as xw,
            tc.tile_pool(name="big", bufs=1) as big,
        ):
            # ---- load constants / inputs ----
            cos_sb = singles.tile([128, T], BF16)
            ssin_sb = singles.tile([128, T], BF16)
            p128_sb = singles.tile([128, 128], BF16)
            mska_sb = singles.tile([128, 4 * QC], BF16)
            mskb_sb = singles.tile([128, 4 * QC], BF16)
            e2_sb = singles.tile([2, 128], BF16)
            nc.sync.dma_start(out=cos_sb, in_=cos2[:])
            nc.sync.dma_start(out=ssin_sb, in_=ssin[:])
            nc.sync.dma_start(out=p128_sb, in_=p128[:])
            nc.sync.dma_start(out=mska_sb, in_=mska[:])
            nc.sync.dma_start(out=mskb_sb, in_=mskb[:])
            nc.sync.dma_start(out=e2_sb, in_=e2[:])

            xT_sb = []
            for kb in range(KB):
                t_ = xw.tile([128, T], BF16, name=f"xT{kb}")
                nc.sync.dma_start(out=t_, in_=xT[kb * 128 : (kb + 1) * 128, :])
                xT_sb.append(t_)
            wq_sb, wk_sb, wv_sb = [], [], []
            for nm, dram, lst in (("wq", wq, wq_sb), ("wk", wk, wk_sb), ("wv", wv, wv_sb)):
                for kb in range(KB):
                    t_ = xw.tile([128, CPC], BF16, name=f"{nm}{kb}")
                    nc.sync.dma_start(out=t_, in_=dram[kb * 128 : (kb + 1) * 128, :])
                    lst.append(t_)
            wc_sb = []
            for cb in range(NT):
                t_ = xw.tile([128, C], BF16, name=f"wc{cb}")
                nc.sync.dma_start(out=t_, in_=wc[cb * 128 : (cb + 1) * 128, :])
                wc_sb.append(t_)

            # ---- persistent big tiles ----
            qt_sb = [big.tile([128, T], BF16, name=f"qt{i}") for i in range(NT)]
            kt_sb = [big.tile([128, T], BF16, name=f"kt{i}") for i in range(NT)]
            yt_sb = [big.tile([128, T], BF16, name=f"yt{i}") for i in range(NT)]
            vp_sb = [big.tile([128, HPC * 65], BF16, name=f"vp{tt}") for tt in range(TT16)]
            # sums staging: engine writes land on aligned partitions {0,32,64,96},
            # then small SBUF->SBUF DMAs (no partition alignment rules) regroup.
            stage4 = big.tile([128, 2 * NQC * QC], BF16, name="stage4")
            sums_sb = big.tile([64, QC], BF16, name="sums")
            sinv2_sb = big.tile([2, NT * NQC * QC], BF16, name="sinv2")

            # ---- projections: QT / KT with RoPE ----
            with (
                tc.tile_pool(name="pj_psum", bufs=4, space="PSUM") as pj_psum,
                tc.tile_pool(name="pj_tmp", bufs=4) as pj_tmp,
            ):
                for w_sb, dst in ((wq_sb, qt_sb), (wk_sb, kt_sb)):
                    for i in range(NT):
                        for tc4 in range(T // 512):
                            ts = slice(tc4 * 512, (tc4 + 1) * 512)
                            ps = pj_psum.tile([128, 512], F32, name="pj")
                            for kb in range(KB):
                                nc.tensor.matmul(
                                    ps,
                                    lhsT=w_sb[kb][:, i * 128 : (i + 1) * 128],
                                    rhs=xT_sb[kb][:, ts],
                                    start=(kb == 0),
                                    stop=(kb == KB - 1),
                                )
                            raw = pj_tmp.tile([128, 512], BF16, name="raw")
                            nc.scalar.copy(out=raw, in_=ps)
                            t1 = pj_tmp.tile([128, 512], BF16, name="t1")
                            nc.vector.tensor_mul(t1, raw, cos_sb[:, ts])
                            # rot(q) via partition-shifted single-input ops
                            # (walrus allows shifted bases only for 1-input)
                            rot_sb = pj_tmp.tile([128, 512], BF16, name="rotsb")
                            for rb in (0, 64):
                                nc.vector.tensor_scalar_mul(
                                    rot_sb[rb : rb + 32, :],
                                    raw[rb + 32 : rb + 64, :],
                                    -1.0,
                                )
                                nc.vector.tensor_copy(
                                    out=rot_sb[rb + 32 : rb + 64, :],
                                    in_=raw[rb : rb + 32, :],
                                )
                            t2 = pj_tmp.tile([128, 512], BF16, name="t2")
                            nc.vector.tensor_mul(t2, rot_sb, ssin_sb[:, ts])
                            nc.vector.tensor_add(dst[i][:, ts], t1, t2)

                # ---- V projection into interleaved V' (65 cols/head) ----
                for tt in range(TT16):
                    ps = pj_psum.tile([128, 512], F32, name="pj")
                    for kb in range(KB):
                        nc.tensor.matmul(
                            ps,
                            lhsT=xT_sb[kb][:, tt * 128 : (tt + 1) * 128],
                            rhs=wv_sb[kb][:, :],
                            start=(kb == 0),
                            stop=(kb == KB - 1),
                        )
                    vdst = vp_sb[tt].rearrange("p (h e) -> p h e", e=65)
                    nc.scalar.copy(
                        out=vdst[:, :, 0:64],
                        in_=ps.rearrange("p (h e) -> p h e", e=64),
                    )
                    nc.vector.memset(vdst[:, :, 64:65], 1.0)

            if phases >= 2:
                # ---- attention: scores^T -> exp -> mask -> PV ----
                with (
                    tc.tile_pool(name="st_psum", bufs=2, space="PSUM") as st_psum,
                    tc.tile_pool(name="pv_psum", bufs=1, space="PSUM") as pv_psum,
                    tc.tile_pool(name="probs", bufs=4) as probs_pool,
                    tc.tile_pool(name="qbd", bufs=4) as qbd_pool,
                ):
                    for g in range(attn_g):  # head groups of 4 (pairs 2g, 2g+1)
                        for qc in range(attn_qc):
                            q0 = qc * QC
                            nkc = (qc + 1) * (QC // KC)
                            # block-diagonal Q chunks, one per pair, reused
                            # across all key chunks j
                            qbd = []
                            for pl in range(2):
                                p = 2 * g + pl
                                qb = qbd_pool.tile([128, 2 * QC], BF16, name=f"qbd{pl}")
                                nc.gpsimd.memset(qb[0:64, QC : 2 * QC], 0.0)
                                nc.gpsimd.memset(qb[64:128, 0:QC], 0.0)
                                nc.vector.tensor_copy(
                                    out=qb[0:64, 0:QC], in_=qt_sb[p][0:64, q0 : q0 + QC]
                                )
                                nc.vector.tensor_copy(
                                    out=qb[64:128, QC : 2 * QC],
                                    in_=qt_sb[p][64:128, q0 : q0 + QC],
                                )
                                qbd.append(qb)
                            # one PSUM bank (512 f32) per head so the four
                            # accumulation groups never share a zero region
                            pv = pv_psum.tile([65, 4, 512], F32, name="pv")
                            for j in range(nkc):
                                st = st_psum.tile([128, 4 * QC], F32, name="st")
                                for pl in range(2):
                                    p = 2 * g + pl
                                    nc.tensor.matmul(
                                        st[:, pl * 2 * QC : (pl + 1) * 2 * QC],
                                        lhsT=kt_sb[p][:, j * KC : (j + 1) * KC],
                                        rhs=qbd[pl],
                                        start=True,
                                        stop=True,
                                    )
                                pr = probs_pool.tile([128, 4 * QC], BF16, name="pr")
                                nc.scalar.activation(
                                    out=pr, in_=st,
                                    func=mybir.ActivationFunctionType.Exp, scale=0.125,
                                )
                                if j == nkc - 2:
                                    nc.vector.tensor_mul(pr, pr, mska_sb)
                                elif j == nkc - 1:
                                    nc.vector.tensor_mul(pr, pr, mskb_sb)
                                for hh in range(4):
                                    h = g * 4 + hh
                                    nc.tensor.matmul(
                                        pv[:, hh, 0:QC],
                                        lhsT=vp_sb[j][:, h * 65 : h * 65 + 65],
                                        rhs=pr[:, hh * QC : (hh + 1) * QC],
                                        start=(j == 0),
                                        stop=(j == nkc - 1),
                                    )
                            for hh in range(4):
                                h = g * 4 + hh
                                ro = (h % 2) * 64
                                nc.vector.tensor_copy(
                                    out=yt_sb[h // 2][ro : ro + 64, q0 : q0 + QC],
                                    in_=pv[0:64, hh, 0:QC],
                                )
                                # sums row -> aligned partition 32*(h%4), unique cols
                                sp = 32 * (h % 4)
                                sc = ((h // 4) * NQC + qc) * QC
                                nc.vector.tensor_copy(
                                    out=stage4[sp : sp + 1, sc : sc + QC],
                                    in_=pv[64:65, hh, 0:QC],
                                )
                                r = qc * 8 + h
                                nc.sync.dma_start(
                                    out=sums_sb[r : r + 1, :],
                                    in_=stage4[sp : sp + 1, sc : sc + QC],
                                )

            if phases >= 3:
                # ---- normalize Y^T and output projection ----
                with (
                    tc.tile_pool(name="bc_psum", bufs=2, space="PSUM") as bc_psum,
                    tc.tile_pool(name="o_psum", bufs=2, space="PSUM") as o_psum,
                    tc.tile_pool(name="o_tmp", bufs=4) as o_tmp,
                    tc.tile_pool(name="sinvp", bufs=1) as sinvp,
                ):
                    sinv_sb = sinvp.tile([64, QC], BF16)
                    with nc.allow_low_precision(reason="softmax denominators tolerate bf16"):
                        nc.vector.reciprocal(out=sinv_sb, in_=sums_sb)
                    # scatter [64, QC] rows (qc*8 + 2i + p) -> [2, (i*NQC+qc)*QC + c]
                    for i in range(NT):
                        for qc in range(NQC):
                            r = qc * 8 + 2 * i
                            s0 = (i * NQC + qc) * QC
                            nc.sync.dma_start(
                                out=sinv2_sb[0:2, s0 : s0 + QC],
                                in_=sinv_sb[r : r + 2, :],
                            )
                    for i in range(NT):
                        for qc in range(NQC):
                            bc = bc_psum.tile([128, QC], F32, name="bc")
                            s0 = (i * NQC + qc) * QC
                            nc.tensor.matmul(
                                bc, lhsT=e2_sb, rhs=sinv2_sb[0:2, s0 : s0 + QC],
                                start=True, stop=True,
                            )
                            bcs = o_tmp.tile([128, QC], BF16, name="bcs")
                            nc.vector.tensor_copy(out=bcs, in_=bc)
                            ts = slice(qc * QC, (qc + 1) * QC)
                            nc.vector.tensor_mul(yt_sb[i][:, ts], yt_sb[i][:, ts], bcs)

                    for qt in range(TT16):
                        for co in range(2):
                            ps = o_psum.tile([128, 512], F32, name="op")
                            for cb in range(NT):
                                nc.tensor.matmul(
                                    ps,
                                    lhsT=yt_sb[cb][:, qt * 128 : (qt + 1) * 128],
                                    rhs=wc_sb[cb][:, co * 512 : (co + 1) * 512],
                                    start=(cb == 0),
                                    stop=(cb == NT - 1),
                                )
                            st_ = o_tmp.tile([128, 512], F32, name="ost")
                            nc.scalar.copy(out=st_, in_=ps)
                            nc.sync.dma_start(
                                out=out[qt * 128 : (qt + 1) * 128, co * 512 : (co + 1) * 512],
                                in_=st_,
                            )
    if split_waits:
        _split_sync_waits(nc)
    return nc


_NC = None


def _host_tables():
    inv_freq = 1.0 / (ROPE_BASE ** (np.arange(0, D, 2, dtype=np.float32) / D))
    t = np.arange(T, dtype=np.float32)
    freqs = np.einsum("i,j->ij", t, inv_freq)          # [T, 32]
    emb = np.concatenate([freqs, freqs], axis=-1)      # [T, 64]
    cosT = np.cos(emb).T.astype(np.float32)            # [64, T]
    sinT = np.sin(emb).T.astype(np.float32)
    cos2 = np.concatenate([cosT, cosT], axis=0)        # [128, T]
    ssin = np.concatenate([sinT, sinT], axis=0)        # [128, T]

    # rotate-half permutation as matmul lhsT: out[m] = sum_k P[k, m] * in[k]
    p128 = np.zeros((128, 128), dtype=np.float32)
    for blk in (0, 64):
        for m in range(32):
            p128[blk + m + 32, blk + m] = -1.0      # out[m] = -in[m+32]
            p128[blk + m, blk + m + 32] = 1.0       # out[m+32] = in[m]

    # causal masks on probs^T [128 keys, QC queries], replicated for 4 heads
    i_ = np.arange(KC)[:, None]
    c_ = np.arange(QC)[None, :]
    mska1 = (c_ >= i_).astype(np.float32)           # key chunk aligned at q0
    mskb1 = (c_ >= i_ + 128).astype(np.float32)     # key chunk at q0+128
    mska = np.tile(mska1, (1, 4))
    mskb = np.tile(mskb1, (1, 4))

    e2 = np.zeros((2, 128), dtype=np.float32)
    e2[0, 0:64] = 1.0
    e2[1, 64:128] = 1.0
    return cos2, ssin, p128, mska, mskb, e2


def kernel(x, Wq, Wkv, Wc):
    global _NC, LAST_EXEC_NS, LAST_RESULTS
    x = np.asarray(x, dtype=np.float32)
    Wq = np.asarray(Wq, dtype=np.float32)
    Wkv = np.asarray(Wkv, dtype=np.float32)
    Wc = np.asarray(Wc, dtype=np.float32)

    if _NC is None:
        _NC = _build_nc()

    cos2, ssin, p128, mska, mskb, e2 = _host_tables()
    bf = lambda a: np.ascontiguousarray(a).astype(NP_BF16)

    in_maps = []
    for core in range(N_CORES):
        b, hh = core // 2, core % 2
        h0 = hh * HPC
        cols = slice(h0 * D, h0 * D + CPC)
        vcols = slice(C + h0 * D, C + h0 * D + CPC)
        in_maps.append(
            {
                "xT": bf(x[b].T),
                "wq": bf(Wq[:, cols]),
                "wk": bf(Wkv[:, cols]),
                "wv": bf(Wkv[:, vcols]),
                "wc": bf(Wc[cols.start : cols.stop, :]),
                "cos2": bf(cos2),
                "ssin": bf(ssin),
                "p128": bf(p128),
                "mska": bf(mska),
                "mskb": bf(mskb),
                "e2": bf(e2),
            }
        )

    trace = os.environ.get("BASS_PROF", "0") == "1"
    res = run_bass_kernel_spmd(_NC, in_maps, list(range(N_CORES)), trace=trace)
    LAST_EXEC_NS = res.exec_time_ns
    LAST_RESULTS = res
    y = np.empty((B, T, C), dtype=np.float32)
    for b in range(B):
        y[b] = res.results[2 * b]["out"] + res.results[2 * b + 1]["out"]
    return y



# revision 5
# speedup vs baseline: 1.1419x; 1.1419x over previous
"""Causal self-attention (B=4, T=2048, C=1024, H=16, D=64, RoPE) on 8 trn2 cores.

Sharding: data-parallel over batch (4) x tensor-parallel over head-halves (2).
core = 2*b + hh handles batch b, heads [hh*8, hh*8+8).

v2 design (vs v1): QC=512 query chunks, per-head scores/PV matmuls (one PSUM
bank per output), diagonal key chunks narrowed to the causally-valid query
window (saves ~20us PE + ~20us ACT), exp-only on ACT, PSUM->SBUF copies on
the Pool (gpsimd) engine, softmax denominators via direct reciprocal from the
PSUM ones-row, broadcast via one K=33 matmul per (pair, round), and the
projection matmuls of round r+1 interleaved as "fillers" into round r's
attention stream so the in-order PE never stalls on the ACT exp pipeline.

Per-core structure (all matmuls bf16, fp32 PSUM):
  - QT/KT projection transposed [c_out, t] + RoPE (DVE), V natural with a
    ones column per head (65 cols/head) for free softmax row sums.
  - attention rounds r=0..3 over 512-query chunks; per (pair, j): scores^T
    per head into st [128k, 2, 512], exp (ACT), staircase mask multiply on
    the 4 diagonal chunks (DVE), PV accumulate per head into pv [65, 2, 512].
  - drain: reciprocal of ones-row -> sinv rows {0,32}, yt copies (Pool),
    broadcast matmul (lhsT [33,128]) -> yt normalize (DVE).
  - out projection row-parallel -> partial [T, C] fp32; host sums pairs.
"""

import os

import numpy as np
import ml_dtypes

import concourse.bass as bass
import concourse.mybir as mybir
import concourse.tile as tile
from concourse.bass_utils import run_bass_kernel_spmd

BF16 = mybir.dt.bfloat16
F32 = mybir.dt.float32
NP_BF16 = ml_dtypes.bfloat16

B, T, C = 4, 2048, 1024
H, D = 16, 64
HPC = 8          # heads per core
CPC = HPC * D    # 512 features per core
N_CORES = 8
QC = 512         # query chunk per attention round
KC = 128         # key chunk
NR = T // QC     # 4 rounds
ROPE_BASE = 10000.0

LAST_EXEC_NS = None
LAST_RESULTS = None


def _split_sync_waits(nc):
    """This walrus build accepts at most one sync wait per instruction; hoist
    extra waits onto same-engine NOPs inserted immediately before."""
    ctr = 0
    for bb in nc.main_func.blocks:
        insts = bb.instructions
        new = []
        changed = False
        for inst in insts:
            si = inst.sync_info
            waits = list(si.on_wait or []) if si is not None else []
            if len(waits) > 1:
                changed = True
                for w in waits[:-1]:
                    ctr += 1
                    nop = mybir.InstNoOp(
                        name=f"waitsplit_nop_{ctr}", ins=[], outs=[],
                        engine=inst.engine,
                    )
                    nop.sync_info = mybir.SyncInfo(on_wait=[w], on_update=[])
                    new.append(nop)
                inst.sync_info = mybir.SyncInfo(
                    on_wait=[waits[-1]], on_update=list(si.on_update or [])
                )
            new.append(inst)
        if changed:
            insts[:] = new


def _build_nc(split_waits=True):
    nc = bass.Bass()

    xT = nc.dram_tensor("xT", [C, T], BF16, kind="ExternalInput")
    wq = nc.dram_tensor("wq", [C, CPC], BF16, kind="ExternalInput")
    wk = nc.dram_tensor("wk", [C, CPC], BF16, kind="ExternalInput")
    wv = nc.dram_tensor("wv", [C, CPC], BF16, kind="ExternalInput")
    wc = nc.dram_tensor("wc", [CPC, C], BF16, kind="ExternalInput")
    cos2 = nc.dram_tensor("cos2", [128, T], BF16, kind="ExternalInput")
    ssin = nc.dram_tensor("ssin", [128, T], BF16, kind="ExternalInput")
    # staircase mask mk[i, h, w] = 1 if w >= i else 0 (same for both heads)
    msk = nc.dram_tensor("msk", [128, 2 * QC], BF16, kind="ExternalInput")
    e2m = nc.dram_tensor("e2m", [33, 128], BF16, kind="ExternalInput")
    out = nc.dram_tensor("out", [T, C], F32, kind="ExternalOutput")

    KB = C // 128          # 8 k-blocks over c_in
    NT = CPC // 128        # 4 head-pair tiles
    TT16 = T // 128        # 16 t tiles for V

    with tile.TileContext(nc) as tc:
        with (
            tc.tile_pool(name="singles", bufs=1) as singles,
            tc.tile_pool(name="xw", bufs=1) as xw,
            tc.tile_pool(name="big", bufs=1) as big,
        ):
            # ---- persistent tiles ----
            cos_sb = singles.tile([128, T], BF16)
            ssin_sb = singles.tile([128, T], BF16)
            msk_sb = singles.tile([128, 2 * QC], BF16)
            e2m_sb = singles.tile([33, 128], BF16)
            xT_sb = [xw.tile([128, T], BF16, name=f"xT{kb}") for kb in range(KB)]
            wq_sb = [xw.tile([128, CPC], BF16, name=f"wq{kb}") for kb in range(KB)]
            wk_sb = [xw.tile([128, CPC], BF16, name=f"wk{kb}") for kb in range(KB)]
            wv_sb = [xw.tile([128, CPC], BF16, name=f"wv{kb}") for kb in range(KB)]
            wc_sb = [xw.tile([128, C], BF16, name=f"wc{cb}") for cb in range(NT)]
            qt_sb = [big.tile([128, T], BF16, name=f"qt{i}") for i in range(NT)]
            kt_sb = [big.tile([128, T], BF16, name=f"kt{i}") for i in range(NT)]
            yt_sb = [big.tile([128, T], BF16, name=f"yt{i}") for i in range(NT)]
            vp_sb = [big.tile([128, HPC * 65], BF16, name=f"vp{tt}") for tt in range(TT16)]
            qbd_sb = [big.tile([128, 2, QC], BF16, name=f"qbd{p}") for p in range(NT)]
            sinv_sb = [big.tile([33, QC], BF16, name=f"sinv{p}") for p in range(NT)]

            mskv = msk_sb.rearrange("p (h q) -> p h q", h=2)

            # ---- input DMAs spread across queues ----
            nc.scalar.dma_start(out=cos_sb, in_=cos2[:, :])
            nc.scalar.dma_start(out=ssin_sb, in_=ssin[:, :])
            nc.scalar.dma_start(out=msk_sb, in_=msk[:, :])
            nc.scalar.dma_start(out=e2m_sb, in_=e2m[:, :])
            for kb in range(KB):
                nc.sync.dma_start(out=xT_sb[kb], in_=xT[kb * 128:(kb + 1) * 128, :])
                nc.sync.dma_start(out=wq_sb[kb], in_=wq[kb * 128:(kb + 1) * 128, :])
                nc.gpsimd.dma_start(out=wk_sb[kb], in_=wk[kb * 128:(kb + 1) * 128, :])
                nc.gpsimd.dma_start(out=wv_sb[kb], in_=wv[kb * 128:(kb + 1) * 128, :])
            for cb in range(NT):
                nc.scalar.dma_start(out=wc_sb[cb], in_=wc[cb * 128:(cb + 1) * 128, :])

            # ---- one-time zero/ones inits (Pool) ----
            for p in range(NT):
                qbv = qbd_sb[p]
                nc.gpsimd.memset(qbv[0:64, 1, :], 0.0)
                nc.gpsimd.memset(qbv[64:128, 0, :], 0.0)
                nc.gpsimd.memset(sinv_sb[p], 0.0)
            for tt in range(TT16):
                vdst = vp_sb[tt].rearrange("p (h e) -> p h e", e=65)
                nc.gpsimd.memset(vdst[:, :, 64:65], 1.0)

            with (
                tc.tile_pool(name="pj_psum", bufs=2, space="PSUM") as pj_psum,
                tc.tile_pool(name="st_psum", bufs=2, space="PSUM") as st_psum,
                tc.tile_pool(name="pv_psum", bufs=1, space="PSUM") as pv_psum,
                tc.tile_pool(name="pr_pool", bufs=3) as pr_pool,
                tc.tile_pool(name="tmp", bufs=3) as tmp,
                tc.tile_pool(name="stg", bufs=2) as stg,
            ):
                # ---------- emission helpers ----------
                def proj_qk_group_thunks(w_sb, dst, i, r):
                    """Thunks: 8 matmuls + rope finish for one [128,512] tile."""
                    ts = slice(r * QC, (r + 1) * QC)
                    state = {}

                    def mk_mm(kb):
                        def f():
                            if kb == 0:
                                state["ps"] = pj_psum.tile([128, QC], F32, name="pj")
                            nc.tensor.matmul(
                                state["ps"],
                                lhsT=w_sb[kb][:, i * 128:(i + 1) * 128],
                                rhs=xT_sb[kb][:, ts],
                                start=(kb == 0),
                                stop=(kb == KB - 1),
                            )
                        return f

                    def fin():
                        ps = state["ps"]
                        raw = tmp.tile([128, QC], BF16, name="raw")
                        nc.scalar.copy(out=raw, in_=ps)
                        t1 = tmp.tile([128, QC], BF16, name="t1")
                        nc.vector.tensor_mul(t1, raw, cos_sb[:, ts])
                        rot = tmp.tile([128, QC], BF16, name="rot")
                        for rb in (0, 64):
                            nc.vector.tensor_scalar_mul(
                                rot[rb:rb + 32, :], raw[rb + 32:rb + 64, :], -1.0
                            )
                            nc.vector.tensor_copy(
                                out=rot[rb + 32:rb + 64, :], in_=raw[rb:rb + 32, :]
                            )
                        t2 = tmp.tile([128, QC], BF16, name="t2")
                        nc.vector.tensor_mul(t2, rot, ssin_sb[:, ts])
                        nc.gpsimd.tensor_add(dst[i][:, ts], t1, t2)
                    return [mk_mm(kb) for kb in range(KB)] + [fin]

                def proj_v_group_thunks(tt):
                    state = {}

                    def mk_mm(kb):
                        def f():
                            if kb == 0:
                                state["ps"] = pj_psum.tile([128, QC], F32, name="pj")
                            nc.tensor.matmul(
                                state["ps"],
                                lhsT=xT_sb[kb][:, tt * 128:(tt + 1) * 128],
                                rhs=wv_sb[kb][:, :],
                                start=(kb == 0),
                                stop=(kb == KB - 1),
                            )
                        return f

                    def fin():
                        ps = state["ps"]
                        vdst = vp_sb[tt].rearrange("p (h e) -> p h e", e=65)
                        nc.scalar.copy(
                            out=vdst[:, :, 0:64],
                            in_=ps.rearrange("p (h e) -> p h e", e=64),
                        )
                    return [mk_mm(kb) for kb in range(KB)] + [fin]

                def outproj_group_thunks(qt, co):
                    state = {}

                    def mk_mm(cb):
                        def f():
                            if cb == 0:
                                state["ps"] = pj_psum.tile([128, 512], F32, name="pj")
                            nc.tensor.matmul(
                                state["ps"],
                                lhsT=yt_sb[cb][:, qt * 128:(qt + 1) * 128],
                                rhs=wc_sb[cb][:, co * 512:(co + 1) * 512],
                                start=(cb == 0),
                                stop=(cb == NT - 1),
                            )
                        return f

                    def fin():
                        ps = state["ps"]
                        st_ = stg.tile([128, 512], F32, name="ost")
                        nc.scalar.copy(out=st_, in_=ps)
                        q = (nc.sync if (qt + co) % 2 == 0 else nc.gpsimd)
                        q.dma_start(
                            out=out[qt * 128:(qt + 1) * 128, co * 512:(co + 1) * 512],
                            in_=st_,
                        )
                    return [mk_mm(cb) for cb in range(NT)] + [fin]

                def proj_round_thunks(r):
                    th = []
                    for w_sb, dst in ((wq_sb, qt_sb), (wk_sb, kt_sb)):
                        for i in range(NT):
                            th.extend(proj_qk_group_thunks(w_sb, dst, i, r))
                    for tt in range(4 * r, 4 * r + 4):
                        th.extend(proj_v_group_thunks(tt))
                    return th

                def outproj_round_thunks(r):
                    th = []
                    for qt in range(4 * r, 4 * r + 4):
                        for co in range(2):
                            th.extend(outproj_group_thunks(qt, co))
                    return th

                # ---------- bootstrap: projections for round 0 ----------
                for f in proj_round_thunks(0):
                    f()

                # ---------- attention rounds with fillers ----------
                for r in range(NR):
                    if r < NR - 1:
                        fillers = proj_round_thunks(r + 1)
                    else:
                        fillers = outproj_round_thunks(2)
                    deficit = 0.0  # est ACT ns minus est PE ns

                    q0 = r * QC
                    ts = slice(q0, q0 + QC)
                    njc = 4 * r + 4
                    for pair in range(NT):
                        qbv = qbd_sb[pair]
                        nc.vector.tensor_copy(
                            out=qbv[0:64, 0, :], in_=qt_sb[pair][0:64, ts]
                        )
                        nc.vector.tensor_copy(
                            out=qbv[64:128, 1, :], in_=qt_sb[pair][64:128, ts]
                        )
                        pv = pv_psum.tile([65, 2, QC], F32, name="pv")
                        for j in range(njc):
                            dj = j - 4 * r
                            off = 128 * dj if dj > 0 else 0
                            W = QC - off
                            st = st_psum.tile([128, 2, QC], F32, name="st")
                            for h2 in range(2):
                                nc.tensor.matmul(
                                    st[:, h2, off:],
                                    lhsT=kt_sb[pair][:, j * KC:(j + 1) * KC],
                                    rhs=qbv[:, h2, off:],
                                    start=True,
                                    stop=True,
                                )
                            deficit += (2 * W) * 0.8333 + 185 - 2 * W * 0.4167
                            while deficit > 0 and fillers:
                                fillers.pop(0)()
                                deficit -= 213.0
                            pr = pr_pool.tile([128, 2, QC], BF16, name="pr")
                            nc.scalar.activation(
                                out=pr[:, :, off:], in_=st[:, :, off:],
                                func=mybir.ActivationFunctionType.Exp, scale=0.125,
                            )
                            if dj >= 0:
                                nc.vector.tensor_mul(
                                    pr[:, :, off:], pr[:, :, off:], mskv[:, :, :W]
                                )
                            for h2 in range(2):
                                h = 2 * pair + h2
                                nc.tensor.matmul(
                                    pv[:, h2, off:],
                                    lhsT=vp_sb[j][:, h * 65:h * 65 + 65],
                                    rhs=pr[:, h2, off:],
                                    start=(j == 0),
                                    stop=(j == njc - 1),
                                    skip_group_check=(dj > 0),
                                )
                        # drain pv: denominators + yt + normalize
                        with nc.allow_low_precision(reason="softmax denom bf16"):
                            nc.vector.reciprocal(
                                out=sinv_sb[pair][0:1, :], in_=pv[64:65, 0, :]
                            )
                            nc.vector.reciprocal(
                                out=sinv_sb[pair][32:33, :], in_=pv[64:65, 1, :]
                            )
                        nc.vector.tensor_copy(
                            out=yt_sb[pair][0:64, ts], in_=pv[0:64, 0, :]
                        )
                        nc.vector.tensor_copy(
                            out=yt_sb[pair][64:128, ts], in_=pv[0:64, 1, :]
                        )
                        bcp = pj_psum.tile([128, QC], F32, name="pj")
                        nc.tensor.matmul(
                            bcp, lhsT=e2m_sb, rhs=sinv_sb[pair], start=True, stop=True
                        )
                        bcs = tmp.tile([128, QC], BF16, name="bcs")
                        nc.vector.tensor_copy(out=bcs, in_=bcp)
                        nc.vector.tensor_mul(
                            yt_sb[pair][:, ts], yt_sb[pair][:, ts], bcs
                        )
                    # flush leftover fillers, then the out-proj of round r-1
                    for f in fillers:
                        f()
                    if 0 < r < NR - 1:
                        for f in outproj_round_thunks(r - 1):
                            f()
                # tail: out-proj of the last round
                for f in outproj_round_thunks(NR - 1):
                    f()
    if split_waits:
        _split_sync_waits(nc)
    return nc


_NC = None


def _host_tables():
    inv_freq = 1.0 / (ROPE_BASE ** (np.arange(0, D, 2, dtype=np.float32) / D))
    t = np.arange(T, dtype=np.float32)
    freqs = np.einsum("i,j->ij", t, inv_freq)          # [T, 32]
    emb = np.concatenate([freqs, freqs], axis=-1)      # [T, 64]
    cosT = np.cos(emb).T.astype(np.float32)            # [64, T]
    sinT = np.sin(emb).T.astype(np.float32)
    cos2 = np.concatenate([cosT, cosT], axis=0)        # [128, T]
    ssin = np.concatenate([sinT, sinT], axis=0)        # [128, T]

    # staircase causal mask on probs^T [128 keys, w]: valid iff w >= i
    i_ = np.arange(KC)[:, None]
    w_ = np.arange(QC)[None, :]
    mk1 = (w_ >= i_).astype(np.float32)                # [128, 512]
    msk = np.concatenate([mk1, mk1], axis=1)           # [128, 2*512]

    e2m = np.zeros((33, 128), dtype=np.float32)
    e2m[0, 0:64] = 1.0
    e2m[32, 64:128] = 1.0
    return cos2, ssin, msk, e2m


def kernel(x, Wq, Wkv, Wc):
    global _NC, LAST_EXEC_NS, LAST_RESULTS
    x = np.asarray(x, dtype=np.float32)
    Wq = np.asarray(Wq, dtype=np.float32)
    Wkv = np.asarray(Wkv, dtype=np.float32)
    Wc = np.asarray(Wc, dtype=np.float32)

    if _NC is None:
        _NC = _build_nc()

    cos2, ssin, msk, e2m = _host_tables()
    bf = lambda a: np.ascontiguousarray(a).astype(NP_BF16)

    in_maps = []
    for core in range(N_CORES):
        b, hh = core // 2, core % 2
        h0 = hh * HPC
        cols = slice(h0 * D, h0 * D + CPC)
        vcols = slice(C + h0 * D, C + h0 * D + CPC)
        in_maps.append(
            {
                "xT": bf(x[b].T),
                "wq": bf(Wq[:, cols]),
                "wk": bf(Wkv[:, cols]),
                "wv": bf(Wkv[:, vcols]),
                "wc": bf(Wc[cols.start:cols.stop, :]),
                "cos2": bf(cos2),
                "ssin": bf(ssin),
                "msk": bf(msk),
                "e2m": bf(e2m),
            }
        )

    trace = os.environ.get("BASS_PROF", "0") == "1"
    res = run_bass_kernel_spmd(_NC, in_maps, list(range(N_CORES)), trace=trace)
    LAST_EXEC_NS = res.exec_time_ns
    LAST_RESULTS = res
    y = np.empty((B, T, C), dtype=np.float32)
    for b in range(B):
        y[b] = res.results[2 * b]["out"] + res.results[2 * b + 1]["out"]
    return y


# revision 6
# speedup vs baseline: 1.1931x; 1.0448x over previous
"""Causal self-attention (B=4, T=2048, C=1024, H=16, D=64, RoPE) on 8 trn2 cores.

Sharding: data-parallel over batch (4) x tensor-parallel over head-halves (2).
core = 2*b + hh handles batch b, heads [hh*8, hh*8+8).

v2 design (vs v1): QC=512 query chunks, per-head scores/PV matmuls (one PSUM
bank per output), diagonal key chunks narrowed to the causally-valid query
window (saves ~20us PE + ~20us ACT), exp-only on ACT, PSUM->SBUF copies on
the Pool (gpsimd) engine, softmax denominators via direct reciprocal from the
PSUM ones-row, broadcast via one K=33 matmul per (pair, round), and the
projection matmuls of round r+1 interleaved as "fillers" into round r's
attention stream so the in-order PE never stalls on the ACT exp pipeline.

Per-core structure (all matmuls bf16, fp32 PSUM):
  - QT/KT projection transposed [c_out, t] + RoPE (DVE), V natural with a
    ones column per head (65 cols/head) for free softmax row sums.
  - attention rounds r=0..3 over 512-query chunks; per (pair, j): scores^T
    per head into st [128k, 2, 512], exp (ACT), staircase mask multiply on
    the 4 diagonal chunks (DVE), PV accumulate per head into pv [65, 2, 512].
  - drain: reciprocal of ones-row -> sinv rows {0,32}, yt copies (Pool),
    broadcast matmul (lhsT [33,128]) -> yt normalize (DVE).
  - out projection row-parallel -> partial [T, C] fp32; host sums pairs.
"""

import os

import numpy as np
import ml_dtypes

import concourse.bass as bass
import concourse.mybir as mybir
import concourse.tile as tile
from concourse.bass_utils import run_bass_kernel_spmd

BF16 = mybir.dt.bfloat16
F32 = mybir.dt.float32
NP_BF16 = ml_dtypes.bfloat16

B, T, C = 4, 2048, 1024
H, D = 16, 64
HPC = 8          # heads per core
CPC = HPC * D    # 512 features per core
N_CORES = 8
QC = 512         # query chunk per attention round
KC = 128         # key chunk
NR = T // QC     # 4 rounds
ROPE_BASE = 10000.0

LAST_EXEC_NS = None
LAST_RESULTS = None


def _split_sync_waits(nc):
    """This walrus build accepts at most one sync wait per instruction; hoist
    extra waits onto same-engine NOPs inserted immediately before."""
    ctr = 0
    for bb in nc.main_func.blocks:
        insts = bb.instructions
        new = []
        changed = False
        for inst in insts:
            si = inst.sync_info
            waits = list(si.on_wait or []) if si is not None else []
            if len(waits) > 1:
                changed = True
                for w in waits[:-1]:
                    ctr += 1
                    nop = mybir.InstNoOp(
                        name=f"waitsplit_nop_{ctr}", ins=[], outs=[],
                        engine=inst.engine,
                    )
                    nop.sync_info = mybir.SyncInfo(on_wait=[w], on_update=[])
                    new.append(nop)
                inst.sync_info = mybir.SyncInfo(
                    on_wait=[waits[-1]], on_update=list(si.on_update or [])
                )
            new.append(inst)
        if changed:
            insts[:] = new


def _build_nc(split_waits=True):
    nc = bass.Bass()

    xT = nc.dram_tensor("xT", [C, T], BF16, kind="ExternalInput")
    wq = nc.dram_tensor("wq", [C, CPC], BF16, kind="ExternalInput")
    wk = nc.dram_tensor("wk", [C, CPC], BF16, kind="ExternalInput")
    wv = nc.dram_tensor("wv", [C, CPC], BF16, kind="ExternalInput")
    wc = nc.dram_tensor("wc", [CPC, C], BF16, kind="ExternalInput")
    cos2 = nc.dram_tensor("cos2", [128, T], BF16, kind="ExternalInput")
    ssin = nc.dram_tensor("ssin", [128, T], BF16, kind="ExternalInput")
    # staircase mask mk[i, h, w] = 1 if w >= i else 0 (same for both heads)
    msk = nc.dram_tensor("msk", [128, 2 * QC], BF16, kind="ExternalInput")
    e2m = nc.dram_tensor("e2m", [33, 128], BF16, kind="ExternalInput")
    out = nc.dram_tensor("out", [T, C], F32, kind="ExternalOutput")

    KB = C // 128          # 8 k-blocks over c_in
    NT = CPC // 128        # 4 head-pair tiles
    TT16 = T // 128        # 16 t tiles for V

    with tile.TileContext(nc) as tc:
        with (
            tc.tile_pool(name="singles", bufs=1) as singles,
            tc.tile_pool(name="xw", bufs=1) as xw,
            tc.tile_pool(name="big", bufs=1) as big,
        ):
            # ---- persistent tiles ----
            cos_sb = singles.tile([128, T], BF16)
            ssin_sb = singles.tile([128, T], BF16)
            msk_sb = singles.tile([128, 2 * QC], BF16)
            e2m_sb = singles.tile([33, 128], BF16)
            xT_sb = [xw.tile([128, T], BF16, name=f"xT{kb}") for kb in range(KB)]
            wq_sb = [xw.tile([128, CPC], BF16, name=f"wq{kb}") for kb in range(KB)]
            wk_sb = [xw.tile([128, CPC], BF16, name=f"wk{kb}") for kb in range(KB)]
            wv_sb = [xw.tile([128, CPC], BF16, name=f"wv{kb}") for kb in range(KB)]
            wc_sb = [xw.tile([128, C], BF16, name=f"wc{cb}") for cb in range(NT)]
            qt_sb = [big.tile([128, T], BF16, name=f"qt{i}") for i in range(NT)]
            kt_sb = [big.tile([128, T], BF16, name=f"kt{i}") for i in range(NT)]
            yt_sb = [big.tile([128, T], BF16, name=f"yt{i}") for i in range(NT)]
            vp_sb = [big.tile([128, HPC * 65], BF16, name=f"vp{tt}") for tt in range(TT16)]
            qbd_sb = [big.tile([128, 2, QC], BF16, name=f"qbd{p}") for p in range(NT)]
            sinv_sb = [big.tile([33, QC], BF16, name=f"sinv{p}") for p in range(NT)]

            mskv = msk_sb.rearrange("p (h q) -> p h q", h=2)

            # ---- input DMAs: interleave so the first Q-proj group's deps
            # land first; sync (SP) and scalar (ACT) are the two HWDGE queues
            nc.scalar.dma_start(out=cos_sb, in_=cos2[:, :])
            nc.scalar.dma_start(out=ssin_sb, in_=ssin[:, :])
            for kb in range(KB):
                nc.sync.dma_start(out=xT_sb[kb], in_=xT[kb * 128:(kb + 1) * 128, :])
                nc.sync.dma_start(out=wq_sb[kb], in_=wq[kb * 128:(kb + 1) * 128, :])
                nc.scalar.dma_start(out=wk_sb[kb], in_=wk[kb * 128:(kb + 1) * 128, :])
                nc.scalar.dma_start(out=wv_sb[kb], in_=wv[kb * 128:(kb + 1) * 128, :])
            nc.scalar.dma_start(out=msk_sb, in_=msk[:, :])
            nc.scalar.dma_start(out=e2m_sb, in_=e2m[:, :])
            for cb in range(NT):
                nc.scalar.dma_start(out=wc_sb[cb], in_=wc[cb * 128:(cb + 1) * 128, :])

            with (
                tc.tile_pool(name="pj_psum", bufs=2, space="PSUM") as pj_psum,
                tc.tile_pool(name="st_psum", bufs=2, space="PSUM") as st_psum,
                tc.tile_pool(name="pv_psum", bufs=1, space="PSUM") as pv_psum,
                tc.tile_pool(name="pr_pool", bufs=3) as pr_pool,
                tc.tile_pool(name="tmp", bufs=3) as tmp,
                tc.tile_pool(name="stg", bufs=2) as stg,
            ):
                # ---------- emission helpers ----------
                def proj_qk_group_thunks(w_sb, dst, i, r):
                    """Thunks: 8 matmuls + rope finish for one [128,512] tile."""
                    ts = slice(r * QC, (r + 1) * QC)
                    state = {}

                    def mk_mm(kb):
                        def f():
                            if kb == 0:
                                state["ps"] = pj_psum.tile([128, QC], F32, name="pj")
                            nc.tensor.matmul(
                                state["ps"],
                                lhsT=w_sb[kb][:, i * 128:(i + 1) * 128],
                                rhs=xT_sb[kb][:, ts],
                                start=(kb == 0),
                                stop=(kb == KB - 1),
                            )
                        return f

                    def fin():
                        ps = state["ps"]
                        raw = tmp.tile([128, QC], BF16, name="raw")
                        nc.scalar.copy(out=raw, in_=ps)
                        t1 = tmp.tile([128, QC], BF16, name="t1")
                        nc.vector.tensor_mul(t1, raw, cos_sb[:, ts])
                        rot = tmp.tile([128, QC], BF16, name="rot")
                        for rb in (0, 64):
                            nc.vector.tensor_scalar_mul(
                                rot[rb:rb + 32, :], raw[rb + 32:rb + 64, :], -1.0
                            )
                            nc.vector.tensor_copy(
                                out=rot[rb + 32:rb + 64, :], in_=raw[rb:rb + 32, :]
                            )
                        t2 = tmp.tile([128, QC], BF16, name="t2")
                        nc.vector.tensor_mul(t2, rot, ssin_sb[:, ts])
                        nc.gpsimd.tensor_add(dst[i][:, ts], t1, t2)
                    return [mk_mm(kb) for kb in range(KB)] + [fin]

                def proj_v_group_thunks(tt):
                    state = {}

                    def mk_mm(kb):
                        def f():
                            if kb == 0:
                                state["ps"] = pj_psum.tile([128, QC], F32, name="pj")
                            nc.tensor.matmul(
                                state["ps"],
                                lhsT=xT_sb[kb][:, tt * 128:(tt + 1) * 128],
                                rhs=wv_sb[kb][:, :],
                                start=(kb == 0),
                                stop=(kb == KB - 1),
                            )
                        return f

                    def fin():
                        ps = state["ps"]
                        vdst = vp_sb[tt].rearrange("p (h e) -> p h e", e=65)
                        nc.scalar.copy(
                            out=vdst[:, :, 0:64],
                            in_=ps.rearrange("p (h e) -> p h e", e=64),
                        )
                    return [mk_mm(kb) for kb in range(KB)] + [fin]

                def outproj_group_thunks(qt, co):
                    state = {}

                    def mk_mm(cb):
                        def f():
                            if cb == 0:
                                state["ps"] = pj_psum.tile([128, 512], F32, name="pj")
                            nc.tensor.matmul(
                                state["ps"],
                                lhsT=yt_sb[cb][:, qt * 128:(qt + 1) * 128],
                                rhs=wc_sb[cb][:, co * 512:(co + 1) * 512],
                                start=(cb == 0),
                                stop=(cb == NT - 1),
                            )
                        return f

                    def fin():
                        ps = state["ps"]
                        st_ = stg.tile([128, 512], F32, name="ost")
                        if (qt + co) % 2 == 0:
                            nc.scalar.copy(out=st_, in_=ps)
                        else:
                            nc.vector.tensor_copy(out=st_, in_=ps)
                        nc.sync.dma_start(
                            out=out[qt * 128:(qt + 1) * 128, co * 512:(co + 1) * 512],
                            in_=st_,
                        )
                    return [mk_mm(cb) for cb in range(NT)] + [fin]

                def proj_round_thunks(r):
                    th = []
                    for w_sb, dst in ((wq_sb, qt_sb), (wk_sb, kt_sb)):
                        for i in range(NT):
                            th.extend(proj_qk_group_thunks(w_sb, dst, i, r))
                    for tt in range(4 * r, 4 * r + 4):
                        th.extend(proj_v_group_thunks(tt))
                    return th

                def outproj_round_thunks(r):
                    th = []
                    for qt in range(4 * r, 4 * r + 4):
                        for co in range(2):
                            th.extend(outproj_group_thunks(qt, co))
                    return th

                # ---------- bootstrap: projections for round 0 ----------
                for f in proj_round_thunks(0):
                    f()

                # one-time zero/ones inits, needed just before attention r0
                for p in range(NT):
                    qbv = qbd_sb[p]
                    nc.gpsimd.memset(qbv[0:64, 1, :], 0.0)
                    nc.gpsimd.memset(qbv[64:128, 0, :], 0.0)
                    nc.gpsimd.memset(sinv_sb[p], 0.0)
                for tt in range(TT16):
                    vdst = vp_sb[tt].rearrange("p (h e) -> p h e", e=65)
                    nc.gpsimd.memset(vdst[:, :, 64:65], 1.0)

                # ---------- attention rounds with fillers ----------
                for r in range(NR):
                    if r < NR - 1:
                        fillers = proj_round_thunks(r + 1)
                    else:
                        fillers = outproj_round_thunks(2)
                    deficit = 0.0  # est ACT ns minus est PE ns
                    pending_norm = []

                    q0 = r * QC
                    ts = slice(q0, q0 + QC)
                    njc = 4 * r + 4
                    for pair in range(NT):
                        qbv = qbd_sb[pair]
                        nc.vector.tensor_copy(
                            out=qbv[0:64, 0, :], in_=qt_sb[pair][0:64, ts]
                        )
                        nc.vector.tensor_copy(
                            out=qbv[64:128, 1, :], in_=qt_sb[pair][64:128, ts]
                        )
                        pv = pv_psum.tile([65, 2, QC], F32, name="pv")
                        for j in range(njc):
                            dj = j - 4 * r
                            off = 128 * dj if dj > 0 else 0
                            W = QC - off
                            st = st_psum.tile([128, 2, QC], F32, name="st")
                            for h2 in range(2):
                                nc.tensor.matmul(
                                    st[:, h2, off:],
                                    lhsT=kt_sb[pair][:, j * KC:(j + 1) * KC],
                                    rhs=qbv[:, h2, off:],
                                    start=True,
                                    stop=True,
                                )
                            deficit += (2 * W) * 0.8333 + 185 - 2 * W * 0.4167
                            while deficit > 0 and fillers:
                                fillers.pop(0)()
                                deficit -= 213.0
                            pr = pr_pool.tile([128, 2, QC], BF16, name="pr")
                            nc.scalar.activation(
                                out=pr[:, :, off:], in_=st[:, :, off:],
                                func=mybir.ActivationFunctionType.Exp, scale=0.125,
                            )
                            if dj >= 0:
                                nc.vector.tensor_mul(
                                    pr[:, :, off:], pr[:, :, off:], mskv[:, :, :W]
                                )
                            for h2 in range(2):
                                h = 2 * pair + h2
                                nc.tensor.matmul(
                                    pv[:, h2, off:],
                                    lhsT=vp_sb[j][:, h * 65:h * 65 + 65],
                                    rhs=pr[:, h2, off:],
                                    start=(j == 0),
                                    stop=(j == njc - 1),
                                    skip_group_check=(dj > 0),
                                )
                        # drain pv: denominators + yt + normalize
                        with nc.allow_low_precision(reason="softmax denom bf16"):
                            nc.vector.reciprocal(
                                out=sinv_sb[pair][0:1, :], in_=pv[64:65, 0, :]
                            )
                            nc.vector.reciprocal(
                                out=sinv_sb[pair][32:33, :], in_=pv[64:65, 1, :]
                            )
                        nc.vector.tensor_copy(
                            out=yt_sb[pair][0:64, ts], in_=pv[0:64, 0, :]
                        )
                        nc.vector.tensor_copy(
                            out=yt_sb[pair][64:128, ts], in_=pv[0:64, 1, :]
                        )
                        def mk_norm(pair, ts):
                            def f():
                                bcp = pj_psum.tile([128, QC], F32, name="pj")
                                nc.tensor.matmul(
                                    bcp, lhsT=e2m_sb, rhs=sinv_sb[pair],
                                    start=True, stop=True,
                                )
                                bcs = tmp.tile([128, QC], BF16, name="bcs")
                                nc.vector.tensor_copy(out=bcs, in_=bcp)
                                nc.vector.tensor_mul(
                                    yt_sb[pair][:, ts], yt_sb[pair][:, ts], bcs
                                )
                            return f
                        pending_norm.append(mk_norm(pair, ts))
                        if len(pending_norm) > 1:
                            pending_norm.pop(0)()
                    # flush leftover fillers, then the out-proj of round r-1
                    for f in fillers:
                        f()
                    for f in pending_norm:
                        f()
                    pending_norm.clear()
                    if 0 < r < NR - 1:
                        for f in outproj_round_thunks(r - 1):
                            f()
                # tail: out-proj of the last round
                for f in outproj_round_thunks(NR - 1):
                    f()
    if split_waits:
        _split_sync_waits(nc)
    return nc


_NC = None


def _host_tables():
    inv_freq = 1.0 / (ROPE_BASE ** (np.arange(0, D, 2, dtype=np.float32) / D))
    t = np.arange(T, dtype=np.float32)
    freqs = np.einsum("i,j->ij", t, inv_freq)          # [T, 32]
    emb = np.concatenate([freqs, freqs], axis=-1)      # [T, 64]
    cosT = np.cos(emb).T.astype(np.float32)            # [64, T]
    sinT = np.sin(emb).T.astype(np.float32)
    cos2 = np.concatenate([cosT, cosT], axis=0)        # [128, T]
    ssin = np.concatenate([sinT, sinT], axis=0)        # [128, T]

    # staircase causal mask on probs^T [128 keys, w]: valid iff w >= i
    i_ = np.arange(KC)[:, None]
    w_ = np.arange(QC)[None, :]
    mk1 = (w_ >= i_).astype(np.float32)                # [128, 512]
    msk = np.concatenate([mk1, mk1], axis=1)           # [128, 2*512]

    e2m = np.zeros((33, 128), dtype=np.float32)
    e2m[0, 0:64] = 1.0
    e2m[32, 64:128] = 1.0
    return cos2, ssin, msk, e2m


def kernel(x, Wq, Wkv, Wc):
    global _NC, LAST_EXEC_NS, LAST_RESULTS
    x = np.asarray(x, dtype=np.float32)
    Wq = np.asarray(Wq, dtype=np.float32)
    Wkv = np.asarray(Wkv, dtype=np.float32)
    Wc = np.asarray(Wc, dtype=np.float32)

    if _NC is None:
        _NC = _build_nc()

    cos2, ssin, msk, e2m = _host_tables()
    bf = lambda a: np.ascontiguousarray(a).astype(NP_BF16)

    in_maps = []
    for core in range(N_CORES):
        b, hh = core // 2, core % 2
        h0 = hh * HPC
        cols = slice(h0 * D, h0 * D + CPC)
        vcols = slice(C + h0 * D, C + h0 * D + CPC)
        in_maps.append(
            {
                "xT": bf(x[b].T),
                "wq": bf(Wq[:, cols]),
                "wk": bf(Wkv[:, cols]),
                "wv": bf(Wkv[:, vcols]),
                "wc": bf(Wc[cols.start:cols.stop, :]),
                "cos2": bf(cos2),
                "ssin": bf(ssin),
                "msk": bf(msk),
                "e2m": bf(e2m),
            }
        )

    trace = os.environ.get("BASS_PROF", "0") == "1"
    res = run_bass_kernel_spmd(_NC, in_maps, list(range(N_CORES)), trace=trace)
    LAST_EXEC_NS = res.exec_time_ns
    LAST_RESULTS = res
    y = np.empty((B, T, C), dtype=np.float32)
    for b in range(B):
        y[b] = res.results[2 * b]["out"] + res.results[2 * b + 1]["out"]
    return y
